# revision 140
# baseline (speedup 1.0000x reference)
"""v45 (from v33 baseline, 578.7us -> 540.3us in the TimelineSim model):

- masks applied POST-exp on pt in SBUF (Pool affine_select triangle zero /
  memset) instead of -1e30 psum adds on DVE: the score->exp->PV path no
  longer shares an in-order engine queue with the rope, which removed the
  ~4.4us PE stalls at every cached-chunk boundary.  Masked pair's exp only
  covers the live 3/4 (dead quarter is memset).
- wo is a generator of single-matmul pieces woven into Act-bound attention
  stretches (exp is 612ns/pair vs 428ns of PE work): each chunk's first
  supertile's wo weaves into the second's attention (from head 1; norm
  pops run 2x so the at tiles are issued in time), and for st_b in {1,3,9}
  (groups whose RS fire waits on later sts anyway) the wo carries into the
  NEXT chunk's first attention.
- chunk 0's projection runs as interleaved half-kt passes over all three
  waves (K/V, Q01, Q23 over kt 0:16, then 16:32; Q3's psum borrows the
  third "sc" slot) so each x tile is consumed three times before the next
  must arrive: the ramp is DMA-supply-bound.
- host pre-packs wq/wkv/wo to [128, kt, o] so every weight DMA descriptor
  is >=1KB (sub-512B descriptors pay 2x DMA latency); wk+wv share one
  tensor/tile; first x tile split so the first matmul waits on 128KB.
- qt in a 2-deep per-chunk ring (attention for chunk N runs in stage N+1)
  freeing 16KB/partition of SBUF -> 3 partial-store buffers.
- ReduceScatter groups (0-3),(8-11),(4,5),(6,7): the merged tail group is
  one 28.1us collective instead of two 21.6us ones; tail partial stores
  stream out in quarters; out_slice copies staggered two fires behind.
- K/V psum evac via DVE (Act's queue is clogged at boundaries), rope sin
  pre-shuffled on host so both rope muls read psum directly (Pool).

Projections/RoPE per 512-column s-chunk, then attention + wo for the two
supertiles it unlocks (chunk order 0,1,4,5,2,3).  All DRAM traffic bf16;
PSUM f32.  PSUM budget (8 banks): "big" x3 (proj waves + wo pp ring),
"sc" x3 (score pairs), "ot" x2 (PV bank + V-transpose staging).
"""
import numpy as np
import ml_dtypes

import concourse.bass as bass
import concourse.mybir as mybir
import concourse.tile as tile
from concourse import bacc
from concourse.bass_utils import run_bass_kernel_spmd
from concourse.masks import make_identity

F32 = mybir.dt.float32
F32R = mybir.dt.float32r
BF16 = mybir.dt.bfloat16

N_CORES = 8
S = 3072
D = 4096
HD = 128
HQ = 4
QO = HQ * HD
KC = 2048
NEG = -1.0e30
EXP_BIAS = -20.0
N_KT = D // 128

SC_ORDER = (0, 1, 4, 5, 2, 3)      # s-chunk processing order
# each chunk unlocks the two supertiles covering its 512 queries
SC_STS = {0: (0, 1), 1: (2, 3), 4: (8, 9), 5: (10, 11), 2: (4, 5), 3: (6, 7)}
RS_GROUPS = [(0, 1, 2, 3), (8, 9, 10, 11), (4, 5), (6, 7)]

_NC_CACHE = {}


def _nvis(st):
    if st < 4:
        return 2 * st + 2
    if st < 8:
        return 8
    return 2 * (st - 8) + 10


def _masks(st):
    if 4 <= st < 8:
        return []
    kb = 2 * st if st < 4 else 2 * st - 8
    return [(0, kb, 0), (0, kb + 1, 1), (1, kb + 1, 0)]


def build_kernel():
    nc = bacc.Bacc("TRN2", target_bir_lowering=False, debug=False,
                   num_devices=N_CORES)

    # weights pre-packed on host to [128, kt, o] so DMA descriptors are
    # per-partition contiguous (>=1KB; <512B pays a 2x DMA latency penalty)
    xT = nc.dram_tensor("xT", [D, S], BF16, kind="ExternalInput").ap()
    wqH = nc.dram_tensor("wqH", [128, N_KT, QO], BF16,
                         kind="ExternalInput").ap()
    wkvH = nc.dram_tensor("wkvH", [128, N_KT, 2, HD], BF16,
                          kind="ExternalInput").ap()
    woH = nc.dram_tensor("woH", [128, HQ, D], BF16,
                         kind="ExternalInput").ap()
    csq = nc.dram_tensor("csq", [HD, 2, S], BF16, kind="ExternalInput").ap()
    out_slice = nc.dram_tensor("out_slice", [S // N_CORES, D], BF16,
                               kind="ExternalOutput").ap()

    partials = [nc.dram_tensor(f"partial{g}", [256 * len(sts), D], BF16).ap()
                for g, sts in enumerate(RS_GROUPS)]
    rs_outs = [nc.dram_tensor(f"rs_out{g}", [32 * len(sts), D], BF16).ap()
               for g, sts in enumerate(RS_GROUPS)]

    shuf_mask = [j ^ 1 for j in range(32)]

    st_group = {}
    for g, sts in enumerate(RS_GROUPS):
        for i, st in enumerate(sts):
            st_group[st] = (g, i)

    with tile.TileContext(nc) as tc:
        with tc.tile_pool(name="glob", bufs=1) as gpool, \
             tc.tile_pool(name="wts", bufs=1) as wpool, \
             tc.tile_pool(name="xp", bufs=9) as xpool, \
             tc.tile_pool(name="csp", bufs=2) as cspool, \
             tc.tile_pool(name="rtmp", bufs=2) as tpool, \
             tc.tile_pool(name="ptp", bufs=2) as ppool, \
             tc.tile_pool(name="smal", bufs=6) as spool, \
             tc.tile_pool(name="atp", bufs=2) as atpool, \
             tc.tile_pool(name="pop", bufs=3) as popool, \
             tc.tile_pool(name="qtp", bufs=2) as qtpool, \
             tc.tile_pool(name="psum", bufs=1, space="PSUM") as psp:
            ident = gpool.tile([128, 128], F32)
            make_identity(nc, ident[:])
            identb = gpool.tile([128, 128], BF16)
            nc.vector.tensor_copy(identb[:], ident[:])
            ones_b = gpool.tile([128, 1], BF16)
            nc.gpsimd.memset(ones_b[:], 1.0)
            bias_t = gpool.tile([128, 1], F32)
            nc.gpsimd.memset(bias_t[:], EXP_BIAS)
            kt_res = gpool.tile([128, KC], BF16)
            v_res = gpool.tile([128, KC // 128, HD], BF16)
            # qt lives in a 2-deep ring: attention for chunk N runs during
            # stage N+1, so only the current+previous chunk's Q is ever live
            chunk_qt = {}

            # weights: chunked loads so first matmuls start early
            wkv_sb = wpool.tile([128, N_KT, 2, HD], BF16)
            wk_sb = wkv_sb[:, :, 0]
            wv_sb = wkv_sb[:, :, 1]
            # first loads issued inside proj_part(ci=0): xt0 first (longest
            # pole for the first matmul), then wkv trickles in 4-8kt pieces
            wq_sb = wpool.tile([128, N_KT, QO], BF16)
            wo_sb = wpool.tile([128, HQ, D], BF16)

            done_sts = set()
            fired = set()
            pending_norm = []
            deferred_out = []
            carry_wo = []
            def proj_part(ci, sc, cs_sb, pf_xts):
                cached = sc in (0, 1, 4, 5)
                scol = slice(sc * 512, (sc + 1) * 512)
                cos_sb = cs_sb[:, 0]
                sin_sb = cs_sb[:, 1]


                # ---- projections, in <=3-head PSUM waves ----
                if cached:
                    waves = [[4, 5], [0, 1], [2, 3]]
                else:
                    waves = [[0, 1], [2, 3]]
                if ci == 0:
                    # issue order tuned for the serialized DMA pool: the
                    # first K/V-wave matmuls need only wk/wv[0:4]+xt0; wq
                    # trickles behind the x tiles (Q waves start at ~18us);
                    # cs0 before rope (~20us); wo only by first attn (~70us)
                    xts = []
                    for ktg in range(N_KT // 4):
                        xt = xpool.tile([128, 4, 512], BF16, tag="xt")
                        xr = xT.rearrange("(kt p) s -> p kt s", p=128)
                        if ktg == 0:
                            # split so the very first matmul only waits on
                            # a 128KB piece (+ the 128KB wkv head)
                            nc.sync.dma_start(xt[:, 0:1],
                                              xr[:, 0:1, scol])
                            nc.sync.dma_start(wkv_sb[:, 0:4], wkvH[:, 0:4])
                            nc.sync.dma_start(xt[:, 1:4],
                                              xr[:, 1:4, scol])
                        else:
                            nc.sync.dma_start(
                                xt[:], xr[:, ktg * 4:(ktg + 1) * 4, scol])
                        xts.append(xt)
                        # demand order under half-kt passes: K/V kt0-15,
                        # Q01 kt0-15, K/V kt16-31, Q01 kt16-31, Q23
                        if ktg == 1:
                            nc.sync.dma_start(wkv_sb[:, 4:8], wkvH[:, 4:8])
                        if ktg == 2:
                            nc.sync.dma_start(wkv_sb[:, 8:16], wkvH[:, 8:16])
                        if ktg == 3:
                            nc.sync.dma_start(wq_sb[:, 0:4, :],
                                              wqH[:, 0:4, :])
                        if ktg == 4:
                            nc.sync.dma_start(wq_sb[:, 4:8, :],
                                              wqH[:, 4:8, :])
                        if ktg == 5:
                            nc.sync.dma_start(wq_sb[:, 8:16, :],
                                              wqH[:, 8:16, :])
                        if ktg == 6:
                            nc.sync.dma_start(wkv_sb[:, 16:24],
                                              wkvH[:, 16:24])
                        if ktg == 7:
                            nc.sync.dma_start(wkv_sb[:, 24:32],
                                              wkvH[:, 24:32])
                            nc.sync.dma_start(cs_sb[:], csq[:, :, 0:512])
                            nc.sync.dma_start(wq_sb[:, 16:32, :],
                                              wqH[:, 16:32, :])
                else:
                    xts = pf_xts
                qt_c = qtpool.tile([128, HQ, 512], BF16, tag="qt")
                chunk_qt[sc] = qt_c
                rope_tiles = {}
                if ci == 0:
                    # chunk 0 is x-supply-bound: split the first two waves
                    # into half-kt passes so every x tile is consumed twice
                    # before the next is needed (halves the demand rate)
                    passes = [(0, 0, 16), (1, 0, 16), (2, 0, 16),
                              (0, 16, 32), (1, 16, 32), (2, 16, 32)]
                else:
                    passes = [(wi, 0, 32) for wi in range(len(waves))]
                wave_psums = {}
                for (wi, k0, k1) in passes:
                    wave = waves[wi]
                    if k0 == 0:
                        psums = {}
                        for i in wave:
                            # K/V wave allocates from the "sc" ring: those
                            # slots free after the last exp, not after wo
                            # evacs, so the next chunk's projections start
                            # without stalling
                            tag = ("sc" if (cached and wi == 0)
                                   or (ci == 0 and i == 3) else "big")
                            psums[i] = psp.tile([128, 512], F32, tag=tag,
                                                bufs=3, name=f"proj{i}")
                        wave_psums[wi] = psums
                    psums = wave_psums[wi]
                    for kt in range(k0, k1):
                        ktg, ktl = kt // 4, kt % 4
                        for i in wave:
                            if i < HQ:
                                w = wq_sb[:, kt, i * 128:(i + 1) * 128]
                            elif i == HQ:
                                w = wk_sb[:, kt, :]
                            else:
                                w = wv_sb[:, kt, :]
                            nc.tensor.matmul(psums[i][:], w,
                                             xts[ktg][:, ktl],
                                             start=(kt == 0),
                                             stop=(kt == N_KT - 1))
                    if k1 < N_KT:
                        continue
                    # rope / V evacuation for this wave.  muls run straight
                    # off psum (no Act copy), sin is pre-shuffled on host so
                    # the shuffle comes AFTER its mul.  The whole rope lives
                    # on DVE: attention's mask adds go to Pool, so a chunk's
                    # rope queue never blocks the previous chunk's attention
                    # (engines are in-order FIFOs)
                    for i in wave:
                        if i < 5:
                            t1 = tpool.tile([128, 512], BF16, tag="t1")
                            nc.vector.tensor_mul(t1[:], psums[i][:], cos_sb)
                            m2 = tpool.tile([128, 512], BF16, tag="t2")
                            nc.vector.tensor_mul(m2[:], psums[i][:], sin_sb)
                            shuf = tpool.tile([128, 512], BF16, tag="shuf")
                            nc.vector.stream_shuffle(shuf[:], m2[:],
                                                     shuf_mask)
                            if i < HQ:
                                nc.vector.tensor_add(qt_c[:, i, :],
                                                     t1[:], shuf[:])
                            else:
                                kcol = sc * 512 if sc < 2 else (sc - 2) * 512
                                nc.vector.tensor_add(
                                    kt_res[:, kcol:kcol + 512], t1[:],
                                    shuf[:])
                        else:
                            vt = tpool.tile([128, 512], BF16, tag="vt")
                            nc.scalar.copy(out=vt[:], in_=psums[i][:])
                            vps = psp.tile([128, 512], BF16, tag="ot",
                                           bufs=2, name="vps",
                                           padded_shape=[128, 1024])
                            for j in range(4):
                                nc.tensor.transpose(
                                    vps[:, j * 128:(j + 1) * 128],
                                    vt[:, j * 128:(j + 1) * 128], identb[:])
                            vb = sc * 4 if sc < 2 else (sc - 2) * 4
                            nc.vector.tensor_copy(
                                out=v_res[:, vb:vb + 4, :], in_=vps[:])

                # prefetch next chunk's x tiles (+cos/sin) now, ahead of
                # later store DMAs, to dodge SP head-of-line blocking
                nxt = None
                if ci + 1 < len(SC_ORDER):
                    nsc = SC_ORDER[ci + 1]
                    nscol = slice(nsc * 512, (nsc + 1) * 512)
                    ncs = cspool.tile([128, 2, 512], BF16, tag="cs")
                    nc.sync.dma_start(ncs[:], csq[:, :, nscol])
                    pf = []
                    for ktg in range(N_KT // 4):
                        xt = xpool.tile([128, 4, 512], BF16, tag="xt")
                        nc.sync.dma_start(
                            xt[:],
                            xT.rearrange("(kt p) s -> p kt s", p=128)[
                                :, ktg * 4:(ktg + 1) * 4, nscol])
                        pf.append(xt)
                    nxt = (ncs, pf)
                if ci == 0:
                    # wo first needed by the first attn_part (~70us):
                    # behind chunk 1's x prefetch, off the critical path
                    nc.sync.dma_start(wo_sb[:], woH[:])
                return nxt

            def attn_part(ci, sc):
                # ---- attention for the two supertiles this chunk unlocks,
                # then wo for both (wo delayed so normalize chains hide) ----
                sts = SC_STS[sc]
                st_at = {}

                def attn_one(st, bg=None, bg_h=2):
                    nvis = _nvis(st)
                    lq = (st - 2 * sc) * 256
                    at_tiles = []
                    for h in range(HQ):
                        qt_slice = chunk_qt[sc][:, h, lq:lq + 256]
                        pt_sb = ppool.tile([128, 16, 256], BF16, tag="pt")
                        # one bank per head: PV cols [0:256], sums [256:258],
                        # transposed reciprocals (bf16) in cols [384:512]
                        ot_ps = psp.tile([128, 512], F32, tag="ot",
                                         bufs=2, name="ot")
                        masks = _masks(st)
                        # masked diagonal pair first (shortens the head-tail
                        # chain); masking happens POST-exp on pt in SBUF via
                        # Pool (affine_select triangle zero / memset), so no
                        # engine in the score->exp path shares a queue with
                        # the rope (Pool is idle at chunk boundaries)
                        pair_order = list(range(0, nvis, 2))
                        if masks:
                            pair_order = [pair_order[-1]] + pair_order[:-1]
                        fm_kb = {mkb for (_, mkb, kind) in masks
                                 if kind == 1}
                        for pi, kb0 in enumerate(pair_order):
                            scT = psp.tile([128, 2, 256], F32, tag="sc",
                                           bufs=3, name="scT")
                            for j in range(2):
                                kb = kb0 + j
                                if kb in fm_kb:
                                    # q-half 0 is fully masked: skip compute;
                                    # exp of the stale psum half is finite
                                    # (old scores) and memset to 0 below
                                    nc.tensor.matmul(
                                        scT[:, j, 128:256],
                                        kt_res[:, kb * 128:(kb + 1) * 128],
                                        qt_slice[:, 128:256],
                                        start=True, stop=True)
                                else:
                                    nc.tensor.matmul(
                                        scT[:, j],
                                        kt_res[:, kb * 128:(kb + 1) * 128],
                                        qt_slice, start=True, stop=True)
                            if masks and pi == 0:
                                # masked pair: exp only the live 3/4 (the
                                # dead quarter is memset below); unblocks
                                # this pair's PV ~300ns earlier
                                nc.scalar.activation(
                                    pt_sb[:, kb0, :], scT[:, 0],
                                    mybir.ActivationFunctionType.Exp,
                                    bias=bias_t[:], scale=1.0)
                                nc.scalar.activation(
                                    pt_sb[:, kb0 + 1, 128:256],
                                    scT[:, 1, 128:256],
                                    mybir.ActivationFunctionType.Exp,
                                    bias=bias_t[:], scale=1.0)
                            else:
                                nc.scalar.activation(
                                    pt_sb[:, kb0:kb0 + 2, :], scT[:],
                                    mybir.ActivationFunctionType.Exp,
                                    bias=bias_t[:], scale=1.0)
                            for (mqi, mkb, kind) in masks:
                                if mkb not in (kb0, kb0 + 1):
                                    continue
                                dst = pt_sb[:, mkb,
                                            mqi * 128:(mqi + 1) * 128]
                                if kind == 1:
                                    nc.gpsimd.memset(dst, 0.0)
                                else:
                                    # zero where q < k (transposed causal)
                                    nc.gpsimd.affine_select(
                                        out=dst, in_=dst,
                                        compare_op=mybir.AluOpType.is_ge,
                                        fill=0.0, base=0,
                                        pattern=[[1, 128]],
                                        channel_multiplier=-1)
                            if pi <= 1 and pending_norm:
                                pending_norm.pop(0)()
                                if pending_norm:
                                    pending_norm.pop(0)()
                            for j in range(2):
                                kb = kb0 + j
                                if kb in fm_kb:
                                    # dead pt half is exactly zero; only
                                    # accumulate the live q-half (never the
                                    # chain start: j==0 of pair 0 is unmasked)
                                    nc.tensor.matmul(
                                        ot_ps[:, 128:256], v_res[:, kb, :],
                                        pt_sb[:, kb, 128:256],
                                        start=False,
                                        stop=(pi == len(pair_order) - 1
                                              and j == 1))
                                else:
                                    nc.tensor.matmul(
                                        ot_ps[:, 0:256], v_res[:, kb, :],
                                        pt_sb[:, kb, :],
                                        start=(pi == 0 and j == 0),
                                        stop=(pi == len(pair_order) - 1
                                              and j == 1))
                            # attention here is Act-bound (612ns exp vs
                            # 428ns of PE per pair): weave in another
                            # supertile's wo pieces to keep PE busy
                            if bg is not None and h >= bg_h:
                                next(bg, None)
                        # softmax sums: P-stationary 1-row matmuls, two
                        # sequential chains (one per q-half) in the PV bank
                        for qh in range(2):
                            for kb in range(nvis):
                                nc.tensor.matmul(
                                    ot_ps[:, 256 + qh:257 + qh],
                                    pt_sb[:, kb, qh * 128:(qh + 1) * 128],
                                    ones_b[:],
                                    start=(kb == 0), stop=(kb == nvis - 1))
                        recip_sb = spool.tile([128, 2], BF16, tag="rcp",
                                              name="recip")
                        with nc.allow_low_precision(reason="bf16 recip"):
                            nc.vector.reciprocal(recip_sb[:],
                                                 ot_ps[:, 256:258])
                        at_sb = atpool.tile([128, 256], BF16, tag=f"at{h}",
                                            name="at")

                        def norm_chain(recip_sb=recip_sb, ot_ps=ot_ps,
                                       at_sb=at_sb):
                            rcT = ot_ps[0:1, 384:512].bitcast(BF16)
                            nc.tensor.transpose(rcT[0:1, 0:128],
                                                recip_sb[:, 0:1], identb[:])
                            nc.tensor.transpose(rcT[0:1, 128:256],
                                                recip_sb[:, 1:2], identb[:])
                            rc_sb = spool.tile([1, 256], BF16, tag="rsb",
                                               name="rc_sb")
                            nc.vector.tensor_copy(out=rc_sb[:], in_=rcT[:])
                            bc = spool.tile([128, 256], BF16, tag="bc",
                                            name="bc")
                            nc.gpsimd.partition_broadcast(bc[:], rc_sb[:])
                            nc.vector.tensor_mul(at_sb[:], ot_ps[:, 0:256],
                                                 bc[:])

                        pending_norm.append(norm_chain)
                        at_tiles.append(at_sb)
                    st_at[st] = at_tiles

                def wo_gen(st):
                    # yields one (j, oc) wo piece per next(): 4 matmuls +
                    # psum evac (+ partial-store fragments), so pieces can
                    # be woven into the sibling supertile's attention
                    at_tiles = st_at[st]
                    g, gi = st_group[st]
                    for j in range(2):
                        po_sb = popool.tile([128, D], BF16, tag="po",
                                            name="po")
                        row = (2 * gi + j) * 128
                        for oc in range(8):
                            pp = psp.tile([128, 512], F32, tag="big",
                                          bufs=3, name="pp")
                            for h in range(HQ):
                                nc.tensor.matmul(
                                    pp[:],
                                    at_tiles[h][:, j * 128:(j + 1) * 128],
                                    wo_sb[:, h, oc * 512:(oc + 1) * 512],
                                    start=(h == 0), stop=(h == HQ - 1))
                                if h < HQ - 1:
                                    yield
                            # the first ~4 oc pieces are the ones woven into
                            # Act-bound attention: keep their evacs off Act
                            # (no exp collisions); the drained remainder
                            # alternates as before
                            if oc % 2 == 0 or (j == 0 and oc < 4):
                                nc.vector.tensor_copy(
                                    out=po_sb[:, oc * 512:(oc + 1) * 512],
                                    in_=pp[:])
                            else:
                                nc.scalar.copy(
                                    out=po_sb[:, oc * 512:(oc + 1) * 512],
                                    in_=pp[:])
                            # tail sts only: stream quarters out as their
                            # evacs land so the last store->ReduceScatter
                            # latency is a quarter store (elsewhere the
                            # extra DMAs' HWDGE slots cost more than saved)
                            if (oc in (1, 3, 5)
                                    and st in SC_STS[SC_ORDER[-1]]):
                                c0 = (oc - 1) * 512
                                nc.sync.dma_start(
                                    partials[g][row:row + 128,
                                                c0:c0 + 1024],
                                    po_sb[:, c0:c0 + 1024])
                            if oc == 6 and st in SC_STS[SC_ORDER[-1]]:
                                nc.sync.dma_start(
                                    partials[g][row:row + 128, 3072:3584],
                                    po_sb[:, 3072:3584])
                            if oc < 7:
                                yield
                        if st in SC_STS[SC_ORDER[-1]]:
                            nc.sync.dma_start(
                                partials[g][row:row + 128, 3584:D],
                                po_sb[:, 3584:D])
                        else:
                            nc.sync.dma_start(partials[g][row:row + 128, :],
                                              po_sb[:])
                        yield

                def wo_one(st, gen=None):
                    while pending_norm:
                        pending_norm.pop(0)()
                    if gen is None:
                        gen = wo_gen(st)
                    for _ in gen:
                        pass

                    done_sts.add(st)
                    for g2, gsts in enumerate(RS_GROUPS):
                        if g2 in fired:
                            continue
                        if all(s in done_sts for s in gsts):
                            fired.add(g2)
                            nrows = 32 * len(gsts)
                            off = sum(32 * len(RS_GROUPS[gg])
                                      for gg in range(g2))
                            nc.gpsimd.collective_compute(
                                "ReduceScatter", mybir.AluOpType.add,
                                replica_groups=[list(range(N_CORES))],
                                ins=[partials[g2]], outs=[rs_outs[g2]])
                            deferred_out.append((off, nrows, g2))
                            # two fires later this group's RS is long done:
                            # its out copy can't head-of-line block SP then
                            if len(deferred_out) >= 3:
                                (o2, n2, gg2) = deferred_out[
                                    len(deferred_out) - 3]
                                nc.sync.dma_start(
                                    out_slice[o2:o2 + n2, :], rs_outs[gg2])

                st_a, st_b = sts
                prev = carry_wo.pop() if carry_wo else None
                attn_one(st_a, bg=(prev[1] if prev else None), bg_h=0)
                if prev:
                    wo_one(prev[0], prev[1])
                gen_a = wo_gen(st_a)
                attn_one(st_b, bg=gen_a, bg_h=1)
                wo_one(st_a, gen_a)
                if st_b in (1, 3, 9):
                    # defer this wo into the next chunk's first attention
                    # (safe: these groups' RS fires wait on later sts
                    # anyway); pieces weave into its Act-bound stretches
                    carry_wo.append((st_b, wo_gen(st_b)))
                else:
                    wo_one(st_b)

            cs0 = cspool.tile([128, 2, 512], BF16, tag="cs", name="cs0")
            carry = (cs0, None)
            for ci, sc in enumerate(SC_ORDER):
                carry = proj_part(ci, sc, carry[0], carry[1])
                if ci >= 1:
                    attn_part(ci - 1, SC_ORDER[ci - 1])
            attn_part(len(SC_ORDER) - 1, SC_ORDER[-1])

            for (off, nrows, g2) in deferred_out[-2:]:
                nc.sync.dma_start(out_slice[off:off + nrows, :],
                                  rs_outs[g2])

    nc.compile()
    return nc


def _host_prep(x, wq, wk, wv, wo, freqs):
    bf = ml_dtypes.bfloat16
    xT = np.ascontiguousarray(x[0].T).astype(bf)
    scale = np.float32(HD ** -0.25)
    cos = (np.cos(freqs) * scale).astype(np.float32).T
    sin = (np.sin(freqs) * scale).astype(np.float32).T
    # csq[:,1] holds sin PRE-SHUFFLED (pair-swapped) so the kernel can
    # multiply before the stream-shuffle: t2[p] = (ps*csq1')[p^1] must equal
    # ps[p^1]*([-sin,sin])[p]  =>  csq1'[p] = ([-sin,sin])[p^1] = [sin,-sin]
    csq = np.empty((HD, 2, S), np.float32)
    csq[0::2, 0] = cos
    csq[1::2, 0] = cos
    csq[0::2, 1] = sin
    csq[1::2, 1] = -sin
    csq = csq.astype(bf)
    def pack(wT):
        # [D, O] -> [128, D//128, O] with partition-major contiguity
        o = wT.shape[1]
        return np.ascontiguousarray(
            wT.reshape(N_KT, 128, o).transpose(1, 0, 2)).astype(bf)

    in_maps = []
    for c in range(N_CORES):
        wkp = pack(wk[c * HD:(c + 1) * HD].T)
        wvp = pack(wv[c * HD:(c + 1) * HD].T)
        in_maps.append({
            "xT": xT,
            "csq": csq,
            "wqH": pack(wq[c * QO:(c + 1) * QO].T),
            "wkvH": np.ascontiguousarray(
                np.stack([wkp, wvp], axis=2)),
            "woH": np.ascontiguousarray(
                wo[:, c * QO:(c + 1) * QO].T.reshape(HQ, 128, D)
                .transpose(1, 0, 2)).astype(bf),
        })
    return in_maps


def kernel(x, wq, wk, wv, wo, freqs, start_pos):
    assert int(start_pos) == 0
    x = np.asarray(x, np.float32)
    wq = np.asarray(wq, np.float32)
    wk = np.asarray(wk, np.float32)
    wv = np.asarray(wv, np.float32)
    wo = np.asarray(wo, np.float32)
    freqs = np.asarray(freqs, np.float32)

    if "nc" not in _NC_CACHE:
        _NC_CACHE["nc"] = build_kernel()
    nc = _NC_CACHE["nc"]

    in_maps = _host_prep(x, wq, wk, wv, wo, freqs)
    res = run_bass_kernel_spmd(nc, in_maps, list(range(N_CORES)))
    out = np.empty((S, D), np.float32)
    for c in range(N_CORES):
        piece = np.asarray(res.results[c]["out_slice"]).astype(np.float32)
        off = 0
        for g, sts in enumerate(RS_GROUPS):
            r8 = 32 * len(sts)
            sub = piece[off:off + r8]
            rows = c * r8 + np.arange(r8)
            grows = 256 * np.asarray(sts)[rows // 256] + rows % 256
            out[grows] = sub
            off += r8
    return out.reshape(1, S, D)

